# revision 23
# baseline (speedup 1.0000x reference)
"""Trainium2 Bass kernel for char-CNN: 5-tap conv along word_length + max-pool.

Reference computation (per (batch, sentence) word, shapes B=64 S=256 W=20 E=128):
    y[w, e] = sum_{kh=0..4} x[w + kh - 2, e] * conv_w[kh]     (zero padded)
    out[e]  = max_w y[w, e] + conv_b

Strategy:
  - Data-parallel over 8 NeuronCores: 8 batches (2048 words) per core.
  - Host packs each core's shard to z[(j w)=120, group, e=128] in fp8e3
    (e3m4, 4 mantissa bits): measured end-to-end rel err 9.7e-3 vs the
    2e-2 budget, and half the HBM traffic of fp16.
  - Conv as matmul: stationary lhsT = x6 [K=120 (6 words x 20 w_in),
    M=128 (e)] fp8e3, moving rhs = conv matrix [120, 120] bf16 -> PSUM
    f32.  Matmuls issue every ~N cycles (LDWEIGHTS pipelines), so PE
    sustains ~120 cols / 50ns per 6-word group.
  - max(a,b) = (a+b)/2 + |a-b|/2: the conv matrix emits 60 half-DIFF
    columns (d) then 60 half-SUM columns (s) for the 10 w-pairs of each
    word.  Pipeline per 8-group sub-chunk: PE writes d -> ACT writes
    |d| (f32, exact) into the s bank -> PE accumulates s on top
    (start=False) -> PSUM now holds the 10 pairwise maxes -> one DVE
    reduce_max over k=10 finishes the word.  This halves the PSUM drain
    (only 60 values/group leave PSUM on DVE, 60 on ACT) and replaces
    the 5-op DVE max tree with a single reduce.
  - Emission is software-pipelined (d(i), s(i-1), reduce(i-1)) so PE
    never head-of-line blocks on ACT's abs.
"""

from contextlib import ExitStack

import numpy as np

import concourse.bass as bass
import concourse.mybir as mybir
import concourse.tile as tile
from concourse import bacc
from concourse.tile_rust import add_dep_helper

W = 20  # word length
E = 128  # embedding dim
KH = 5  # conv taps
PAD = 2
J = 6  # words per matmul group (6 * 20 = 120 <= 128 partitions)
KP = J * W  # contraction size / partitions used (120)
KK = W // 2  # w-pairs per word (10)
HC = KK * J  # columns per group per half (60)
NCORES = 8
BANK = 512  # PSUM bank size in f32 elements


def build_conv_matrix(conv_w: np.ndarray) -> np.ndarray:
    """[KP, KP] conv matrix in sum/diff form.  Column blocks:
    cols [0, 60): d(k, j)  = (y(2k, j) - y(2k+1, j)) / 2
    cols [60,120): s(k, j) = (y(2k, j) + y(2k+1, j)) / 2
    where y(w, j) is the plain conv output column (tap band) and columns
    within a block are ordered k*J + j (j contiguous)."""
    import ml_dtypes

    wv = np.asarray(conv_w, np.float32).reshape(-1)
    assert wv.shape == (KH,)
    a = np.zeros((KP, W, J), np.float32)  # [row, w_out, j]
    for j in range(J):
        for wo in range(W):
            for kh in range(KH):
                wi = wo + kh - PAD
                if 0 <= wi < W:
                    a[j * W + wi, wo, j] = wv[kh]
    sd = np.zeros((KP, KP), np.float32)
    for k in range(KK):
        ya, yb = a[:, 2 * k, :], a[:, 2 * k + 1, :]
        sd[:, k * J : (k + 1) * J] = 0.5 * (ya - yb)  # d block
        sd[:, HC + k * J : HC + (k + 1) * J] = 0.5 * (ya + yb)  # s block
    return sd.astype(ml_dtypes.bfloat16)


def pack_input(x_core: np.ndarray, ng: int) -> np.ndarray:
    """[nw, W, E] f32 -> [KP, ng, E] fp8e3 (e3m4) partition-major, zero-
    padded to ng*J words."""
    import ml_dtypes

    nw = x_core.shape[0]
    xp = np.zeros((ng * J, W, E), ml_dtypes.float8_e3m4)
    xp[:nw] = x_core.astype(ml_dtypes.float8_e3m4)
    # (g j) w e -> (j w) g e
    return np.ascontiguousarray(
        xp.reshape(ng, J, W, E).transpose(1, 2, 0, 3).reshape(KP, ng, E)
    )


def chunk_plan(ng: int, big: int = 64) -> list[int]:
    """Descending chunk sizes: big early (fewer ring bubbles while the
    stream is deep), small at the end (short pipeline tail)."""
    sizes = []
    rem = ng
    for sz, keep in ((64, 96), (32, 48), (16, 24), (8, 8)):
        if sz > big:
            continue
        while rem >= max(sz, keep):
            sizes.append(sz)
            rem -= sz
    if rem:
        sizes.append(rem)
    return sizes


def build_nc(
    nw: int,
    dma_rings: tuple[str, ...] = ("sync",),
    bufs: int = 24,
    first_ring: str | None = None,
    big_chunk: int = 64,
    cg: int = 8,
    psum_bufs: int = 4,
    lookahead: int = 2,
    flush_words: int = 512,
) -> bass.Bass:
    """Build the per-core Bass graph. nw = real words per core."""
    f32 = mybir.dt.float32
    bf16 = mybir.dt.bfloat16
    f8 = mybir.dt.float8e3
    ng = (nw + J - 1) // J  # padded group count
    nwp = ng * J  # padded word count

    nc = bacc.Bacc()
    z_ext = nc.declare_dram_parameter("z", [KP, ng, E], f8, isOutput=False)
    a_ext = nc.declare_dram_parameter("a", [KP, KP], bf16, isOutput=False)
    out_ext = nc.declare_dram_parameter("out", [E, nw], bf16, isOutput=True)

    engines = {
        "sync": nc.sync,
        "scalar": nc.scalar,
        "gpsimd": nc.gpsimd,
    }

    with ExitStack() as ctx:
        tc = ctx.enter_context(tile.TileContext(nc))
        const = ctx.enter_context(tc.tile_pool(name="const", bufs=1))
        hpool = ctx.enter_context(tc.tile_pool(name="xh", bufs=bufs))
        opool = ctx.enter_context(tc.tile_pool(name="o", bufs=1))
        pspool = ctx.enter_context(
            tc.tile_pool(name="ps", bufs=psum_bufs, space="PSUM")
        )

        a_t = const.tile([KP, KP], bf16)
        nc.sync.dma_start(out=a_t[:, :], in_=a_ext[:, :])
        maxt = opool.tile([E, nwp], bf16)

        # Prime the "pending zero" state of every pool tile's s bank: a
        # matmul start=True zero-marks its whole 2KB PSUM region, and a
        # start=False write landing on pending-zero bytes OVERWRITES
        # instead of accumulating.  The s-matmuls below rely on
        # accumulate-onto-ACT-data, so clear the pending bits once with a
        # zero matmul (lhsT = zeros) covering the s columns.
        zl = const.tile([KP, E], bf16)
        zr = const.tile([KP, cg * HC], bf16)
        nc.vector.memset(zl[:, :], 0.0)
        nc.vector.memset(zr[:, :], 0.0)
        for _ in range(psum_bufs):
            pt = pspool.tile([E, 2 * BANK], f32, tag="ps")
            nc.tensor.matmul(
                pt[:, BANK : BANK + cg * HC],
                lhsT=zl[:, :],
                rhs=zr[:, :],
                start=False,
                stop=True,
                skip_group_check=True,
            )

        def stage_d(xh, coff, sg0, sn):
            """d-matmuls for one sub-chunk into bank 0 of a fresh tile."""
            ps = pspool.tile([E, 2 * BANK], f32, tag="ps")
            for g in range(sn):
                nc.tensor.matmul(
                    ps[:, g * HC : (g + 1) * HC],
                    lhsT=xh[:, coff + g * E : coff + (g + 1) * E],
                    rhs=a_t[:, 0:HC],
                    start=True,
                    stop=True,
                )
            # ACT: |d| -> s bank (f32, exact); matmul_s accumulates onto it
            abs_i = nc.scalar.activation(
                ps[:, BANK : BANK + sn * HC],
                ps[:, 0 : sn * HC],
                mybir.ActivationFunctionType.Abs,
            )
            return ps, abs_i

        def stage_s(ps, abs_i, xh, coff, sn):
            for g in range(sn):
                mm = nc.tensor.matmul(
                    ps[:, BANK + g * HC : BANK + (g + 1) * HC],
                    lhsT=xh[:, coff + g * E : coff + (g + 1) * E],
                    rhs=a_t[:, HC:KP],
                    start=False,
                    stop=True,
                    skip_group_check=True,
                )
                # the framework does not model start=False as a READ of
                # the |d| data ACT parked in the s bank — add the edge
                add_dep_helper(
                    mm.ins, abs_i.ins, reason="s-matmul accumulates onto abs"
                )

        def stage_r(ps, sg0, sn):
            # s bank now holds s + |d| = the 10 pairwise maxes per word;
            # reduce over k (innermost in AP, stride J in memory).
            pv = ps[:, BANK : BANK + sn * HC].rearrange(
                "p (g k j) -> p g j k", k=KK, j=J
            )
            out_v = maxt[:, sg0 * J : (sg0 + sn) * J].rearrange(
                "p (g j) -> p g j", g=sn
            )
            nc.vector.reduce_max(out_v, pv, axis=mybir.AxisListType.X)

        g0 = 0
        if first_ring is not None:
            sizes = [16] + chunk_plan(ng - 16, big_chunk)
            rings = [first_ring] + [
                dma_rings[i % len(dma_rings)] for i in range(len(sizes) - 1)
            ]
        else:
            if ng > 32:
                # two small warm-up chunks so compute starts sooner
                sizes = [8, 8] + chunk_plan(ng - 16, big_chunk)
            else:
                sizes = chunk_plan(ng, big_chunk)
            rings = [dma_rings[i % len(dma_rings)] for i in range(len(sizes))]
        max_gn = max(sizes)

        # Phase A: the whole input stream is issued up front (bufs covers
        # every chunk) so no compute op can head-of-line-block a DMA
        # trigger.
        subs = []
        for ring, gn in enumerate(sizes):
            eng_name = rings[ring]
            src = z_ext[:, g0 : g0 + gn, :].rearrange("p g e -> p (g e)")
            xh = hpool.tile([KP, max_gn * E], f8, tag="xh")
            engines[eng_name].dma_start(out=xh[:, 0 : gn * E], in_=src)
            for s0 in range(0, gn, cg):
                sn = min(cg, gn - s0)
                subs.append((xh, s0 * E, g0 + s0, sn))
            g0 += gn

        # Phase B: software-pipelined compute.  Emit order per iteration:
        # d(i)+abs(i), then s(i-1), then reduce(i-1) — so PE's s-matmuls
        # never wait on the abs of their own sub-chunk.
        w_flushed = 0

        def flush_out(upto_words):
            nonlocal w_flushed
            hi = min(upto_words, nw)
            if hi - w_flushed >= flush_words or (hi >= nw and hi > w_flushed):
                # flushes ride the otherwise-idle SWDGE ring: a flush whose
                # source isn't computed yet must not head-of-line-block the
                # input chunk triggers (HWDGE queues are FIFO)
                nc.gpsimd.dma_start(
                    out=out_ext[:, w_flushed:hi], in_=maxt[:, w_flushed:hi]
                )
                w_flushed = hi

        from collections import deque

        pend = deque()

        def drain_one():
            pps, pabs, pxh, pcoff, psg0, psn = pend.popleft()
            stage_s(pps, pabs, pxh, pcoff, psn)
            stage_r(pps, psg0, psn)
            flush_out(psg0 * J + psn * J)

        for sub in subs:
            xh, coff, sg0, sn = sub
            ps, abs_i = stage_d(xh, coff, sg0, sn)
            pend.append((ps, abs_i, xh, coff, sg0, sn))
            if len(pend) > lookahead:
                drain_one()
        while pend:
            drain_one()
        flush_out(nw)
    nc.finalize()
    return nc


def kernel(embedded_char, conv_w, conv_b):
    from concourse.bass_utils import run_bass_kernel_spmd

    x = np.asarray(embedded_char, np.float32)
    b_val = float(np.asarray(conv_b, np.float32).reshape(-1)[0])
    B, S, Wl, El = x.shape
    assert (Wl, El) == (W, E)
    bs = B // NCORES
    nw = bs * S
    ng = (nw + J - 1) // J
    a16 = build_conv_matrix(conv_w)

    nc = build_nc(nw)
    in_maps = [
        {
            "z": pack_input(x[i * bs : (i + 1) * bs].reshape(nw, Wl, El), ng),
            "a": a16,
        }
        for i in range(NCORES)
    ]
    res = run_bass_kernel_spmd(nc, in_maps, core_ids=list(range(NCORES)))
    full = np.concatenate(
        [
            r["out"].astype(np.float32).T.reshape(bs, S, El)
            for r in res.results
        ],
        axis=0,
    )
    if b_val != 0.0:
        full = full + b_val
    return np.ascontiguousarray(full.astype(np.float32))
